# revision 3
# baseline (speedup 1.0000x reference)
"""Two-phase Bass/Tile kernels for the Contrast5 loss (SPMD, 8 cores x 3 batches).

Kernel A: unc = sum_c pred*ln(pred+1e-6), streamed to DRAM as bf16.
  Per chunk: DMA-in (SP) -> Ln (Act) -> mult (Pool/GPSIMD, full rate) ->
  channel-reduce (DVE, bf16 out) -> DMA-out (Act engine queue).
Kernel B: normalized contrastive loss partials over gathered candidates,
  one activation table (Ln+Exp combined set), vectorized across batches.
Host: exact top-5 selection (bf16 candidate superset + exact f32 recheck),
  proj gather, final scalar reduction.
"""

import sys
for _p in ("/root/.axon_site/_ro/trn_rl_repo", "/opt/trn_rl_repo"):
    if _p not in sys.path:
        sys.path.append(_p)
import numpy as np
import concourse.bass as bass
import concourse.bacc as bacc
import concourse.mybir as mybir
import concourse.tile as tile

F32 = mybir.dt.float32
BF16 = mybir.dt.bfloat16
U32 = mybir.dt.uint32
AF = mybir.ActivationFunctionType
OP = mybir.AluOpType
AX = mybir.AxisListType

B_LOC = 3
C = 4
HW = 65536
D = 64
S = 5
NI = 3
P = NI - 1
TAU = 0.07
EPS_LOG = 1e-6
EPS_DEN = 1e-8
NCORES = 8
# combined Ln+Exp act table index in act_info.json ordering
ACT_TABLE_LN_EXP = 6

# chunk splits per batch: (start, size) in pixel columns (of 512).
# Small first chunks fill the pipeline fast; small last chunks shrink the tail.
CHUNKS_BY_BATCH = [
    [(0, 128), (128, 128), (256, 256)],
    [(0, 256), (256, 256)],
    [(0, 256), (256, 128), (384, 64), (448, 64)],
]


def dedupe_act_loads(nc, set_id=ACT_TABLE_LN_EXP):
    """Post-compile: collapse greedy per-func table loads into one load of the
    combined Ln/Exp table per block, hoisted to the block start so it runs
    before (not after) the tile scheduler's first DMA-wait event."""
    for blk in nc.m.functions[0].blocks:
        first = None
        for inst in list(blk.instructions):
            if isinstance(inst, mybir.InstLoadActFuncSet):
                si = inst.sync_info
                assert si is None or (not si.on_wait and not si.on_update)
                blk.instructions.remove(inst)
                if first is None:
                    first = inst
                    inst.act_func_set_id = set_id
        if first is not None:
            blk.instructions.insert(0, first)
    return nc


def build_nc_a():
    nc = bacc.Bacc("TRN2", target_bir_lowering=False, debug=False)
    pred_in = nc.dram_tensor("pred", [B_LOC, C, HW], F32, kind="ExternalInput")
    unc_out = nc.dram_tensor("unc", [128, B_LOC * 512], BF16, kind="ExternalOutput")

    with tile.TileContext(nc) as tc:
        with tc.tile_pool(name="sb", bufs=4) as pool, tc.tile_pool(
            name="cst", bufs=1
        ) as cpool:
            eps_bias = cpool.tile([128, 1], F32, tag="eps_bias")
            nc.vector.memset(eps_bias[:], EPS_LOG)
            unc_all = cpool.tile([128, B_LOC * 512], BF16, tag="unc_all")
            for b in range(B_LOC):
                for (x0, xc) in CHUNKS_BY_BATCH[b]:
                    predt = pool.tile([128, C, xc], F32, tag=f"pred{xc}")
                    nc.sync.dma_start(
                        out=predt[:, :, :],
                        in_=pred_in[b].rearrange("c (p x) -> p c x", p=128)[
                            :, :, x0 : x0 + xc
                        ],
                    )
                    logt = pool.tile([128, C, xc], F32, tag=f"logt{xc}")
                    nc.scalar.activation(
                        out=logt[:, :, :], in_=predt[:, :, :], func=AF.Ln,
                        bias=eps_bias[:], scale=1.0,
                    )
                    prodt = pool.tile([128, C, xc], F32, tag=f"prodt{xc}")
                    nc.gpsimd.tensor_tensor(
                        out=prodt[:, :, :], in0=predt[:, :, :], in1=logt[:, :, :],
                        op=OP.mult,
                    )
                    with nc.allow_low_precision(
                        reason="selection values; host rechecks exact f32"
                    ):
                        nc.vector.tensor_reduce(
                            out=unc_all[:, b * 512 + x0 : b * 512 + x0 + xc],
                            in_=prodt[:].rearrange("p c x -> p x c"),
                            op=OP.add, axis=AX.X,
                        )
            # two batched writebacks on SP, issued after all input DMAs
            nc.sync.dma_start(out=unc_out[:, 0:1024], in_=unc_all[:, 0:1024])
            nc.sync.dma_start(out=unc_out[:, 1024:1536], in_=unc_all[:, 1024:1536])
    nc.compile()
    return dedupe_act_loads(nc)


def build_nc_b():
    nc = bacc.Bacc("TRN2", target_bir_lowering=False, debug=False)
    # psel columns: [curr (b,s): 15 | pos (b,i,s): 30]
    psel_in = nc.dram_tensor("psel", [D, NI * B_LOC * S], F32, kind="ExternalInput")
    # consts packed: cols 0:45 maskpos rows 0:15; 45:60 negmask rows 0:15;
    # col 60 ones (all 64 partitions); cols 64:128 ones row (partition 0)
    cst_in = nc.dram_tensor("cst", [D, 128], F32, kind="ExternalInput")
    out_dram = nc.dram_tensor("out", [S * B_LOC, 1], F32, kind="ExternalOutput")

    NCOL = NI * B_LOC * S  # 45
    NR = S * B_LOC  # 15

    with tile.TileContext(nc) as tc:
        with (
            tc.tile_pool(name="sb", bufs=2) as pool,
            tc.tile_pool(name="cst", bufs=1) as cpool,
            tc.tile_pool(name="ps", bufs=1, space="PSUM") as pp,
        ):
            psel = cpool.tile([D, NCOL], F32, tag="psel")
            nc.sync.dma_start(out=psel[:], in_=psel_in[:])
            cst = cpool.tile([D, 128], F32, tag="cst")
            nc.sync.dma_start(out=cst[:], in_=cst_in[:])
            maskpos = cst[0:NR, 0:NCOL]
            negmask = cst[0:NR, NCOL : NCOL + NR]
            ones_col = cst[:, 60:61]
            ones_row = cst[0:1, 64 : 64 + D]

            # ||x||^-1 = exp(-0.5*ln(sum x^2)) : stays on the Ln/Exp table
            sq = pool.tile([D, NCOL], F32, tag="sq")
            nc.vector.tensor_tensor(out=sq[:], in0=psel[:], in1=psel[:], op=OP.mult)
            nrm_ps = pp.tile([1, NCOL], F32, tag="nrm")
            nc.tensor.matmul(nrm_ps[:], lhsT=ones_col, rhs=sq[:], start=True, stop=True)
            lnn = pool.tile([1, NCOL], F32, tag="lnn")
            nc.scalar.activation(out=lnn[:], in_=nrm_ps[:], func=AF.Ln)
            rinv = pool.tile([1, NCOL], F32, tag="rinv")
            nc.scalar.activation(out=rinv[:], in_=lnn[:], func=AF.Exp, scale=-0.5)
            rb_ps = pp.tile([D, NCOL], F32, tag="rb")
            nc.tensor.matmul(
                rb_ps[:], lhsT=ones_row, rhs=rinv[:], start=True, stop=True
            )
            xh = pool.tile([D, NCOL], F32, tag="xh")
            nc.vector.tensor_tensor(out=xh[:], in0=psel[:], in1=rb_ps[:], op=OP.mult)

            # G[r, j] = xh[:, r] . xh[:, j] for the 15 curr columns
            g_ps = pp.tile([NR, NCOL], F32, tag="g")
            nc.tensor.matmul(
                g_ps[:], lhsT=xh[:, 0:NR], rhs=xh[:], start=True, stop=True
            )
            # pos_sim/tau: maskpos is pre-scaled by 1/TAU on the host
            mp = pool.tile([NR, NCOL], F32, tag="mp")
            nc.vector.tensor_tensor(out=mp[:], in0=g_ps[:], in1=maskpos, op=OP.mult)
            pos_sim = pool.tile([NR, 1], F32, tag="pos_sim")
            nc.vector.reduce_sum(out=pos_sim[:], in_=mp[:], axis=AX.X)
            # E = exp(G_curr/tau); neg = sum_{j!=s, same b} E
            em = pool.tile([NR, NR], F32, tag="em")
            nc.scalar.activation(
                out=em[:], in_=g_ps[:, 0:NR], func=AF.Exp, scale=1.0 / TAU
            )
            mn = pool.tile([NR, NR], F32, tag="mn")
            nc.vector.tensor_tensor(out=mn[:], in0=em[:], in1=negmask, op=OP.mult)
            neg = pool.tile([NR, 1], F32, tag="neg")
            nc.vector.reduce_sum(out=neg[:], in_=mn[:], axis=AX.X)
            # contrib = ln(1 + (neg+eps)*exp(-pos_sim/tau))
            #         = ln((pos + neg + eps)/pos)
            em2 = pool.tile([NR, 1], F32, tag="em2")
            nc.scalar.activation(out=em2[:], in_=pos_sim[:], func=AF.Exp, scale=-1.0)
            q = pool.tile([NR, 1], F32, tag="q")
            nc.vector.scalar_tensor_tensor(
                out=q[:], in0=neg[:], scalar=EPS_DEN, in1=em2[:],
                op0=OP.add, op1=OP.mult,
            )
            contrib = pool.tile([NR, 1], F32, tag="contrib")
            nc.scalar.activation(
                out=contrib[:], in_=q[:], func=AF.Ln, bias=ones_col[0:NR, :]
            )
            nc.sync.dma_start(out=out_dram[:], in_=contrib[:])
    nc.compile()
    return dedupe_act_loads(nc)


def host_constants_b():
    cst = np.zeros((D, 128), np.float32)
    for b in range(B_LOC):
        for s in range(S):
            r = b * S + s
            for i in range(P):
                cst[r, 15 + (b * P + i) * S + s] = 1.0 / TAU
            for s2 in range(S):
                if s2 != s:
                    cst[r, 45 + b * S + s2] = 1.0
    cst[:, 60] = 1.0
    cst[0, 64 : 64 + D] = 1.0
    return cst


def host_select(unc_core, pred_core):
    """unc_core: (128, B_LOC*512) bf16-ish device unc; pred_core: (B_LOC,C,HW).
    Top-64 candidate superset by device value, exact f32 recheck -> top-5."""
    chosen = np.empty((B_LOC, S), np.int64)
    u = np.asarray(unc_core, dtype=np.float32)
    u = u.reshape(128, B_LOC, 512).transpose(1, 0, 2).reshape(B_LOC, HW)
    K = 64
    for b in range(B_LOC):
        cand = np.argpartition(-u[b], K)[:K]
        pv = pred_core[b][:, cand]
        exact = (pv * np.log(pv + EPS_LOG)).sum(axis=0)
        top = cand[np.argsort(-exact, kind="stable")[:S]]
        chosen[b] = top
    return chosen


def host_gather(proj, core, chosen):
    """Build psel (64, 45) for one core: cols [curr(b,s) | pos(b,i,s)]."""
    b0 = core * B_LOC
    psel = np.empty((D, NI * B_LOC * S), np.float32)
    for b in range(B_LOC):
        hw = chosen[b]
        psel[:, b * S : (b + 1) * S] = proj[0, b0 + b].reshape(D, HW)[:, hw]
        for i in range(P):
            psel[:, 15 + (b * P + i) * S : 15 + (b * P + i + 1) * S] = proj[
                i + 1, b0 + b
            ].reshape(D, HW)[:, hw]
    return psel


def shard_pred(pred):
    pred_r = np.ascontiguousarray(pred.reshape(24, C, HW))
    return [
        {"pred": pred_r[c * B_LOC : (c + 1) * B_LOC]} for c in range(NCORES)
    ]


# ---------------------------------------------------------------------------
# Harness entry point: kernel(**inputs) -> full-shape output (scalar f32).
# ---------------------------------------------------------------------------
from concourse.bass_utils import run_bass_kernel_spmd

_CACHE = {}


def _get_programs():
    if "a" not in _CACHE:
        _CACHE["a"] = build_nc_a()
        _CACHE["b"] = build_nc_b()
    return _CACHE["a"], _CACHE["b"]


def kernel(pred, proj, mask, pseudo_label, idx, sample_num):
    assert int(idx) == 0 and int(sample_num) == S
    pred = np.ascontiguousarray(np.asarray(pred, dtype=np.float32))
    proj = np.asarray(proj, dtype=np.float32)
    nc_a, nc_b = _get_programs()
    core_ids = list(range(NCORES))

    shards = shard_pred(pred)
    res_a = run_bass_kernel_spmd(nc_a, shards, core_ids=core_ids)

    cst = host_constants_b()
    in_maps_b = []
    for core in range(NCORES):
        chosen = host_select(res_a.results[core]["unc"], shards[core]["pred"])
        psel = host_gather(proj, core, chosen)
        in_maps_b.append({"psel": psel, "cst": cst})

    res_b = run_bass_kernel_spmd(nc_b, in_maps_b, core_ids=core_ids)
    total = np.float32(
        sum(r["out"].ravel().astype(np.float64).sum() for r in res_b.results)
        / (S * 24.0)
    )
    return total.reshape(())
